# revision 15
# baseline (speedup 1.0000x reference)
"""Trainium2 Bass kernel for nn_F0Predictor (conv stack + LSTM decode), 8-core data-parallel.

Contract: kernel(**inputs) takes the FULL unsharded inputs (as produced by
setup_inputs()) and returns the full [128, num_steps, 2] float32 output.
Internally: batch is sharded 8 ways (16 per NeuronCore), weights replicated,
compute in bf16 with fp32 PSUM accumulation. No collectives.

LSTM step design (v2):
  - gates psum P[128,512]: partition 32*hc+b, col 128*g+u, gate order (i,f,o,g)
  - g-gate rows pre-scaled by 2 host-side so one Sigmoid over all 512 cols
    gives sigma(i,f,o) and sigma(2g) (tanh(g) = 2*sigma(2g)-1, folded into the
    DVE chain via scalar_tensor_tensor)
  - x_t (rank-2 + bias) folded into one aux matmul with lhsT rows
    (lf0, sigma(uv), 1)
  - all elementwise state in bf16 -> DVE 2x mode
"""
import numpy as np
import ml_dtypes

import concourse.bass as bass
import concourse.tile as tile
from concourse import bacc, mybir
from concourse.bass_utils import run_bass_kernel_spmd

BF = mybir.dt.bfloat16
F32 = mybir.dt.float32
BF_NP = ml_dtypes.bfloat16
F8 = mybir.dt.float8e4
F8_NP = ml_dtypes.float8_e4m3

NCORES = 8
BC = 16          # batch per core
# torch LSTM row offsets for gate order (i, g, f, o)
TGOFF = [0, 1024, 512, 1536]
Sigmoid = mybir.ActivationFunctionType.Sigmoid
Tanh = mybir.ActivationFunctionType.Tanh
Relu = mybir.ActivationFunctionType.Relu
ALU = mybir.AluOpType

_CACHE = {}


# --------------------------------------------------------------------------
# host-side prep (numpy): weight layout transforms, batch sharding
# --------------------------------------------------------------------------

def _prep(inp):
    f32 = np.float32
    P = {}
    x = np.asarray(inp["x"], f32).reshape(128, 8192)
    x_pad = np.zeros((128, 8224), f32)
    x_pad[:, 16:8208] = x
    T0 = np.stack([x_pad[:, k + 1: k + 1 + 8192: 4] for k in range(31)], 0)  # [31,128,2048]
    P["t0_full"] = T0.astype(BF_NP)

    w0 = np.asarray(inp["cw0"], f32)
    P["w0T"] = w0[:, 0, :].T.astype(BF_NP).copy()                 # [31, 64]
    P["cb0"] = np.asarray(inp["cb0"], f32).reshape(64, 1).copy()

    w1 = np.asarray(inp["cw1"], f32)
    w1p = np.zeros((128, 16, 128), f32)               # [r, kp, co]
    for k in range(16):
        w1p[0:64, k, :] = w1[:, :, 2 * k].T
        if 2 * k + 1 <= 30:
            w1p[64:128, k, :] = w1[:, :, 2 * k + 1].T
    P["w1p"] = w1p.astype(BF_NP)
    P["cb1"] = np.asarray(inp["cb1"], f32).reshape(128, 1).copy()

    w2 = np.asarray(inp["cw2"], f32)
    w2T = np.zeros((128, 31, 2, 128), f32)                        # [r, k, cc, co]
    for k in range(31):
        for cc in range(2):
            w2T[:, k, cc, :] = w2[128 * cc:128 * cc + 128, :, k].T
    P["w2T"] = w2T.astype(BF_NP)
    P["cb2"] = np.ascontiguousarray(np.asarray(inp["cb2"], f32).reshape(2, 128).T)

    w3 = np.asarray(inp["cw3"], f32)
    w3T = np.zeros((128, 31, 2, 4, 128), f32)                     # [r, k, ci, cc, co]
    for k in range(31):
        for ci in range(2):
            for cc in range(4):
                w3T[:, k, ci, cc, :] = w3[128 * cc:128 * cc + 128, 128 * ci:128 * ci + 128, k].T
    P["w3T"] = w3T.astype(BF_NP)
    P["cb3"] = np.ascontiguousarray(np.asarray(inp["cb3"], f32).reshape(4, 128).T)

    w4 = np.asarray(inp["cw4"], f32)
    w4R = np.zeros((31, 4, 128, 1024), f32)                       # [k, ci, r, co]
    for k in range(31):
        for ci in range(4):
            w4R[k, ci] = w4[:, 128 * ci:128 * ci + 128, k].T
    P["w4R"] = (w4R * 16.0).astype(F8_NP)
    P["cb4"] = (np.asarray(inp["cb4"], f32).reshape(1, 1024) * 16.0).astype(BF_NP)

    phw = np.asarray(inp["ph_w"], f32)
    pcw = np.asarray(inp["pc_w"], f32)
    pwT = np.zeros((64, 128, 2, 4, 128), f32)                     # [kk, r, s, hc, uu]
    for kk in range(64):
        for hc in range(4):
            pwT[kk, :, 0, hc, :] = phw[128 * hc:128 * hc + 128, 128 * kk:128 * kk + 128].T
            pwT[kk, :, 1, hc, :] = pcw[128 * hc:128 * hc + 128, 128 * kk:128 * kk + 128].T
    P["pwT"] = (pwT * 16.0).astype(F8_NP)
    pb = np.zeros((1, 2, 4, 128), f32)
    pb[0, 0] = np.asarray(inp["ph_b"], f32).reshape(4, 128)
    pb[0, 1] = np.asarray(inp["pc_b"], f32).reshape(4, 128)
    P["pb"] = (pb * 16.0).astype(BF_NP)

    # ---- LSTM weights, v2 layout -------------------------------------
    whh = np.asarray(inp["w_hh"], f32)
    # wG[r, kk, hc, 128*g + u] = whh[TGOFF[g] + 128*hc + u, 128*kk + r]
    # g-gate block (g==3) pre-scaled by 2 for the sigmoid-only trick.
    wG = np.zeros((128, 4, 4, 512), f32)
    for kk in range(4):
        for hc in range(4):
            for g in range(4):
                blk = whh[TGOFF[g] + 128 * hc: TGOFF[g] + 128 * hc + 128,
                          128 * kk:128 * kk + 128].T   # [r, u]
                wG[:, kk, hc, 128 * g:128 * g + 128] = blk * (2.0 if g == 1 else 1.0)
    P["wG"] = wG.astype(BF_NP)

    wih = np.asarray(inp["w_ih"], f32)
    embw = np.asarray(inp["emb_w"], f32)
    M = wih @ embw                                # [2048, 2]
    const0 = np.asarray(inp["b_ih"], f32) + np.asarray(inp["b_hh"], f32)
    consts = const0 + wih @ np.asarray(inp["emb_b"], f32)
    # mRA rows (lf0 coeff, const); mRB row (uv coeff); [row, s, hc, 128*g + u]
    mRA = np.zeros((2, 2, 4, 512), f32)
    mRB = np.zeros((1, 2, 4, 512), f32)
    for hc in range(4):
        for g in range(4):
            sl = slice(TGOFF[g] + 128 * hc, TGOFF[g] + 128 * hc + 128)
            sc = 2.0 if g == 1 else 1.0
            c = slice(128 * g, 128 * g + 128)
            mRA[0, 1, hc, c] = M[sl, 0] * sc
            mRA[1, 0, hc, c] = const0[sl] * sc
            mRA[1, 1, hc, c] = consts[sl] * sc
            mRB[0, 1, hc, c] = M[sl, 1] * sc
    P["mRA"] = mRA.astype(BF_NP)
    P["mRB"] = mRB.astype(BF_NP)

    hwT = np.zeros((128, 4, 2), f32)
    for kk in range(4):
        hwT[:, kk, 0] = np.asarray(inp["lf0_w"], f32)[0, 128 * kk:128 * kk + 128]
        hwT[:, kk, 1] = np.asarray(inp["uv_w"], f32)[0, 128 * kk:128 * kk + 128]
    P["hwT"] = hwT.astype(BF_NP)
    P["hb2"] = np.array([[np.asarray(inp["lf0_b"], f32).reshape(-1)[0],
                          np.asarray(inp["uv_b"], f32).reshape(-1)[0]]], f32)
    P["i128"] = np.eye(128, dtype=BF_NP)
    return P


# --------------------------------------------------------------------------
# device program
# --------------------------------------------------------------------------

def _build(T):
    nc = bacc.Bacc("TRN2", target_bir_lowering=False, debug=False, num_devices=NCORES)

    d_t0 = nc.dram_tensor("t0", [31, BC, 2048], BF, kind="ExternalInput")
    d_w0 = nc.dram_tensor("w0T", [31, 64], BF, kind="ExternalInput")
    d_cb0 = nc.dram_tensor("cb0", [64, 1], F32, kind="ExternalInput")
    d_w1 = nc.dram_tensor("w1p", [128, 16, 128], BF, kind="ExternalInput")
    d_cb1 = nc.dram_tensor("cb1", [128, 1], F32, kind="ExternalInput")
    d_w2 = nc.dram_tensor("w2T", [128, 31, 2, 128], BF, kind="ExternalInput")
    d_cb2 = nc.dram_tensor("cb2", [128, 2], F32, kind="ExternalInput")
    d_w3 = nc.dram_tensor("w3T", [128, 31, 2, 4, 128], BF, kind="ExternalInput")
    d_cb3 = nc.dram_tensor("cb3", [128, 4], F32, kind="ExternalInput")
    d_w4 = nc.dram_tensor("w4R", [31, 4, 128, 1024], F8, kind="ExternalInput")
    d_cb4 = nc.dram_tensor("cb4", [1, 1024], BF, kind="ExternalInput")
    d_pw = nc.dram_tensor("pwT", [64, 128, 2, 4, 128], F8, kind="ExternalInput")
    d_pb = nc.dram_tensor("pb", [1, 2, 4, 128], BF, kind="ExternalInput")
    d_wG = nc.dram_tensor("wG", [128, 4, 4, 512], BF, kind="ExternalInput")
    d_mRA = nc.dram_tensor("mRA", [2, 2, 4, 512], BF, kind="ExternalInput")
    d_mRB = nc.dram_tensor("mRB", [1, 2, 4, 512], BF, kind="ExternalInput")
    d_oinitA = nc.dram_tensor("oinitA", [2, 16 * (T + 1)], BF, kind="ExternalInput")
    d_oinitB = nc.dram_tensor("oinitB", [1, 16 * (T + 1)], BF, kind="ExternalInput")
    d_hwT = nc.dram_tensor("hwT", [128, 4, 2], BF, kind="ExternalInput")
    d_hb2 = nc.dram_tensor("hb2", [1, 2], F32, kind="ExternalInput")
    d_i128 = nc.dram_tensor("i128", [128, 128], BF, kind="ExternalInput")
    d_out = nc.dram_tensor("out", [2, T, 16], F32, kind="ExternalOutput")
    d_warm = nc.dram_tensor("warm", [1, 16], F32, kind="ExternalOutput")

    from contextlib import ExitStack
    with tile.TileContext(nc) as tc, ExitStack() as top:
        const_pool = top.enter_context(tc.tile_pool(name="const", bufs=1))
        i128t = const_pool.tile([128, 128], BF)
        nc.sync.dma_start(i128t[:], d_i128.ap())
        hb2t = const_pool.tile([1, 2], F32)
        nc.sync.dma_start(hb2t[:], d_hb2.ap())

        # persistent activations for the conv chain
        act1_pool = top.enter_context(tc.tile_pool(name="act1", bufs=1))
        act2_pool = top.enter_context(tc.tile_pool(name="act2", bufs=1))
        act3_pool = top.enter_context(tc.tile_pool(name="act3", bufs=1))
        out4_pool = top.enter_context(tc.tile_pool(name="out4", bufs=1))

        act1 = act1_pool.tile([128, BC, 543], BF)
        nc.gpsimd.memset(act1[:], 0.0)
        act2 = [act2_pool.tile([128, BC, 159], BF, name=f"act2_{i}", tag=f"act2_{i}") for i in range(2)]
        for t_ in act2:
            nc.gpsimd.memset(t_[:], 0.0)
        act3 = [act3_pool.tile([128, BC, 63], BF, name=f"act3_{i}", tag=f"act3_{i}") for i in range(4)]
        for t_ in act3:
            nc.gpsimd.memset(t_[:], 0.0)
        out4T = out4_pool.tile([128, 1024], BF)

        # prefetch L2 weights early (DMA overlaps L0/L1 compute)
        es_w2 = ExitStack()
        p2p = es_w2.enter_context(tc.tile_pool(name="p2", bufs=1))
        w2t = p2p.tile([128, 31, 2, 128], BF)
        nc.sync.dma_start(w2t[:], d_w2.ap())
        cb2t = p2p.tile([128, 2], F32)
        nc.sync.dma_start(cb2t[:], d_cb2.ap())

        # ---------------- L0 + L1 (own pools, freed after) ----------------
        with ExitStack() as es01:
            p01 = es01.enter_context(tc.tile_pool(name="p01", bufs=1))
            ps01 = es01.enter_context(tc.tile_pool(name="ps01", bufs=2, space="PSUM"))
            t0t = p01.tile([31, BC, 2048], BF)
            nc.sync.dma_start(t0t[:], d_t0.ap())
            w0t = p01.tile([31, 64], BF)
            nc.sync.dma_start(w0t[:], d_w0.ap())
            cb0t = p01.tile([64, 1], F32)
            nc.sync.dma_start(cb0t[:], d_cb0.ap())
            act0 = p01.tile([128, BC, 2079], BF)
            nc.gpsimd.memset(act0[:], 0.0)

            for bg in range(4):
                for lc in range(16):
                    p = ps01.tile([64, 4, 128], F32, name="l0ps", tag="l0ps")
                    nc.tensor.matmul(p[:], w0t[:],
                                     t0t[:, 4 * bg:4 * bg + 4, 128 * lc:128 * lc + 128],
                                     start=True, stop=True)
                    nc.scalar.activation(
                        act0[0:64, 4 * bg:4 * bg + 4, 15 + 128 * lc:15 + 128 * lc + 128],
                        p[:], Relu, bias=cb0t[:])
                # duplicate shifted by +1 element into partitions 64..127,
                # chunked per bg so the copy overlaps the next bg's matmuls
                nc.sync.dma_start(act0[64:128, 4 * bg:4 * bg + 4, 0:2078],
                                  act0[0:64, 4 * bg:4 * bg + 4, 1:2079])

            w1t = p01.tile([128, 16, 128], BF)
            nc.sync.dma_start(w1t[:], d_w1.ap())
            cb1t = p01.tile([128, 1], F32)
            nc.sync.dma_start(cb1t[:], d_cb1.ap())

            for bg in range(4):
                for lc in range(4):
                    p1 = ps01.tile([128, 4, 128], F32, name="l1ps", tag="l1ps", bufs=4)
                    for kp in range(16):
                        j0 = 2 * kp + 512 * lc
                        rhs = act0[:, 4 * bg:4 * bg + 4, j0: j0 + 512: 4]
                        nc.tensor.matmul(p1[:], w1t[:, kp, :], rhs,
                                         start=(kp == 0), stop=(kp == 15))
                    nc.scalar.activation(
                        act1[:, 4 * bg:4 * bg + 4, 15 + 128 * lc:15 + 128 * lc + 128],
                        p1[:], Relu, bias=cb1t[:])

        # prefetch L3 weights (DMA overlaps L2 compute)
        es_w3 = ExitStack()
        p3p = es_w3.enter_context(tc.tile_pool(name="p3", bufs=1))
        w3t = p3p.tile([128, 31, 2, 4, 128], BF)
        nc.sync.dma_start(w3t[:], d_w3.ap())
        cb3t = p3p.tile([128, 4], F32)
        nc.sync.dma_start(cb3t[:], d_cb3.ap())

        # ---------------- L2 ----------------
        with ExitStack() as es2:
            ps2 = es2.enter_context(tc.tile_pool(name="ps2", bufs=1, space="PSUM"))
            for cc in range(2):
                p2 = [ps2.tile([128, 4, 128], F32, name=f"l2ps_{bg}", tag=f"l2ps_{bg}") for bg in range(4)]
                for k in range(31):
                    for bg in range(4):
                        rhs = act1[:, 4 * bg:4 * bg + 4, k: k + 512: 4]
                        nc.tensor.matmul(p2[bg][:], w2t[:, k, cc, :], rhs,
                                         start=(k == 0), stop=(k == 30))
                for bg in range(4):
                    nc.scalar.activation(act2[cc][:, 4 * bg:4 * bg + 4, 15:143],
                                         p2[bg][:], Relu, bias=cb2t[:, cc:cc+1])

        # ---------------- L3 ----------------
        with ExitStack() as es3:
            ps3 = es3.enter_context(tc.tile_pool(name="ps3", bufs=2, space="PSUM"))
            for cc in range(4):
                p3 = ps3.tile([128, BC, 32], F32, name="l3ps", tag="l3ps")
                n = 0
                for ci in range(2):
                    for k in range(31):
                        rhs = act2[ci][:, :, k:k + 128:4]
                        nc.tensor.matmul(p3[:], w3t[:, k, ci, cc, :], rhs,
                                         start=(n == 0), stop=(n == 61))
                        n += 1
                nc.scalar.activation(act3[cc][:, :, 15:47], p3[:], Relu, bias=cb3t[:, cc:cc+1])
        es_w3.close()
        es_w2.close()

        # ---------------- L4 (weights moving) ----------------
        with ExitStack() as es4:
            p4p = es4.enter_context(tc.tile_pool(name="p4", bufs=8))
            p4c = es4.enter_context(tc.tile_pool(name="p4c", bufs=1))
            ps4 = es4.enter_context(tc.tile_pool(name="ps4", bufs=1, space="PSUM"))
            ones1 = p4c.tile([1, 128], BF)
            nc.gpsimd.memset(ones1[:], 1.0)
            cb4t = p4c.tile([1, 1024], BF)
            nc.sync.dma_start(cb4t[:], d_cb4.ap())
            PT = [ps4.tile([128, 512], F32, name=f"l4ps_{j}", tag=f"l4ps_{j}") for j in range(2)]
            for j in range(2):
                nc.tensor.matmul(PT[j][:], ones1[:, 0:128], cb4t[:, 512 * j:512 * j + 512],
                                 start=True, stop=False)
            for k in range(31):
                for ci in range(4):
                    w4c = p4p.tile([128, 1024], F8, name="w4c", tag="w4c", bufs=12)
                    nc.sync.dma_start(w4c[:], d_w4.ap()[k, ci])
                    imt = p4p.tile([128, 8, 16], F8, name="imt", tag="imt", bufs=4)
                    nc.vector.tensor_copy(
                        imt[:], act3[ci][:, :, k:k + 32:4].rearrange("p b l -> p l b"))
                    last = (k == 30 and ci == 3)
                    for j in range(2):
                        nc.tensor.matmul(PT[j][:], imt[:], w4c[:, 512 * j:512 * j + 512],
                                         start=False, stop=last)
            for j in range(2):
                nc.scalar.activation(out4T[:, 512 * j:512 * j + 512], PT[j][:], Relu,
                                     scale=1.0 / 16.0)

        # ---------------- transposes + projections ----------------
        lstm_pool = top.enter_context(tc.tile_pool(name="lstm", bufs=1))
        C = lstm_pool.tile([128, 128], BF)
        outA = lstm_pool.tile([2, 16 * (T + 1)], BF)   # rows (lf0, ones)
        outB = lstm_pool.tile([1, 16 * (T + 1)], BF)   # row sig(uv)
        nc.sync.dma_start(outA[:], d_oinitA.ap())
        nc.sync.dma_start(outB[:], d_oinitB.ap())

        state_pool = top.enter_context(tc.tile_pool(name="state", bufs=2))
        ps_tr = top.enter_context(tc.tile_pool(name="ps_tr", bufs=2, space="PSUM"))

        with ExitStack() as esp:
            ppw = esp.enter_context(tc.tile_pool(name="ppw", bufs=8))
            ppc = esp.enter_context(tc.tile_pool(name="ppc", bufs=1))
            psp = esp.enter_context(tc.tile_pool(name="psp", bufs=1, space="PSUM"))
            hfT = ppc.tile([128, 1024], F8)
            # transpose out4T[l*16+b, co] -> hfT[:, 16*kk+b] (kk = l*8 + c8),
            # two l-values per [32,128] transpose (base partitions 0/32/64/96)
            for q in range(4):
                ptile = ps_tr.tile([128, 8, 2, 16], BF, name="trp2", tag="trp")
                for c8 in range(8):
                    nc.tensor.transpose(
                        ptile[:, c8, :, :],
                        out4T[32 * q:32 * q + 32, 128 * c8:128 * c8 + 128],
                        i128t[32 * q:32 * q + 32, 32 * q:32 * q + 32],
                        tile_position=(32 * q, 0))
                dst = hfT[:, 256 * q:256 * q + 256].rearrange(
                    "p (l cc b) -> p cc l b", l=2, cc=8, b=16)
                nc.scalar.copy(dst, ptile[:])

            onesb = ppc.tile([1, 16], BF)
            nc.gpsimd.memset(onesb[:], 1.0)
            pbt = ppc.tile([1, 2, 4, 128], BF)
            nc.sync.dma_start(pbt[:], d_pb.ap())
            psh = [psp.tile([128, 128], F32, name=f"psh_{s}", tag=f"psh_{s}") for s in range(2)]
            for s in range(2):
                for hc in range(4):
                    nc.tensor.matmul(psh[s][32 * hc:32 * hc + BC, :], onesb[:],
                                     pbt[:, s, hc, :], start=True, stop=False,
                                     tile_position=(0, 32 * hc))
            for kk in range(64):
                pwc = ppw.tile([128, 2, 4, 128], F8, name="pwc", tag="pwc", bufs=16)
                nc.sync.dma_start(pwc[:], d_pw.ap()[kk])
                last = (kk == 63)
                for s in range(2):
                    for hc in range(4):
                        nc.tensor.matmul(psh[s][32 * hc:32 * hc + BC, :],
                                         hfT[:, 16 * kk:16 * kk + 16],
                                         pwc[:, s, hc, :], start=False, stop=last,
                                         tile_position=(0, 32 * hc))
            Hb0 = state_pool.tile([128, 128], BF, name="Hb", tag="Hb")
            nc.scalar.mul(Hb0[:], psh[0][:], 1.0 / 16.0)
            nc.scalar.mul(C[:], psh[1][:], 1.0 / 16.0)

        # ---------------- LSTM ----------------
        wGt = lstm_pool.tile([128, 4, 4, 512], BF)
        nc.sync.dma_start(wGt[:], d_wG.ap())
        mRAt = lstm_pool.tile([2, 2, 4, 512], BF)
        nc.sync.dma_start(mRAt[:], d_mRA.ap())
        mRBt = lstm_pool.tile([1, 2, 4, 512], BF)
        nc.sync.dma_start(mRBt[:], d_mRB.ap())
        hwTt = lstm_pool.tile([128, 4, 2], BF)
        nc.sync.dma_start(hwTt[:], d_hwT.ap())

        ps_g = top.enter_context(tc.tile_pool(name="ps_g", bufs=1, space="PSUM"))
        ps_hd = top.enter_context(tc.tile_pool(name="ps_hd", bufs=1, space="PSUM"))
        work_pool = top.enter_context(tc.tile_pool(name="work", bufs=2))

        def trans_h(hb):
            pt = ps_tr.tile([128, 128], BF, name="trp", tag="trp")
            nc.tensor.transpose(pt[:], hb[:], i128t[:])
            hTT = state_pool.tile([128, 128], BF, name="hTT", tag="hTT")
            nc.scalar.copy(hTT[:], pt[:])
            return hTT

        hTT = trans_h(Hb0)
        # HAM warm-up: >3.4us of dense matmuls so the LSTM runs at 2.4 GHz.
        # Output written to a junk DRAM tensor so the burst is not DCE'd.
        wu = ps_g.tile([128, 384], F32, name="P1", tag="P1", bufs=1)
        for r in range(16):
            for hc in range(4):
                nc.tensor.matmul(wu[32 * hc:32 * hc + BC, :], hTT[:, 0:16],
                                 wGt[:, r % 4, hc, 0:384],
                                 start=(r == 0), stop=(r == 15),
                                 tile_position=(0, 32 * hc))
        wscr = work_pool.tile([1, 16], F32, name="wscr", tag="wscr")
        nc.vector.tensor_copy(wscr[:], wu[0:1, 0:16])
        nc.sync.dma_start(d_warm.ap(), wscr[:])
        # transpose initial C into CT (cell state kept in transposed layout)
        ptc = ps_tr.tile([128, 128], BF, name="trp0", tag="trp")
        nc.tensor.transpose(ptc[:], C[:], i128t[:])
        CT = lstm_pool.tile([128, 128], BF)
        nc.vector.tensor_copy(CT[:], ptc[:])

        for t in range(T):
            s_idx = 0 if t == 0 else 1
            SA = outA[:, 16 * t:16 * t + 16]
            SB = outB[:, 16 * t:16 * t + 16]
            # bank 1: (i, g', f) gate columns — finishes early so the whole
            # sigmoid/DVE chain overlaps bank 2's (o-gate) streams
            P1 = ps_g.tile([128, 384], F32, name="P1", tag="P1", bufs=1)
            for kk in range(4):
                for hc in range(4):
                    nc.tensor.matmul(P1[32 * hc:32 * hc + BC, :],
                                     hTT[:, 32 * kk:32 * kk + 16],
                                     wGt[:, kk, hc, 0:384],
                                     start=(kk == 0), stop=False,
                                     tile_position=(0, 32 * hc))
            for hc in range(4):
                nc.tensor.matmul(P1[32 * hc:32 * hc + BC, :], SA,
                                 mRAt[:, s_idx, hc, 0:384],
                                 start=False, stop=False,
                                 tile_position=(0, 32 * hc))
            for hc in range(4):
                nc.tensor.matmul(P1[32 * hc:32 * hc + BC, :], SB,
                                 mRBt[:, s_idx, hc, 0:384],
                                 start=False, stop=True,
                                 tile_position=(0, 32 * hc))
            # bank 2: (o)
            P2 = ps_g.tile([128, 128], F32, name="P2", tag="P2", bufs=1)
            for kk in range(4):
                for hc in range(4):
                    nc.tensor.matmul(P2[32 * hc:32 * hc + BC, :],
                                     hTT[:, 32 * kk:32 * kk + 16],
                                     wGt[:, kk, hc, 384:512],
                                     start=(kk == 0), stop=False,
                                     tile_position=(0, 32 * hc))
            for hc in range(4):
                nc.tensor.matmul(P2[32 * hc:32 * hc + BC, :], SA,
                                 mRAt[:, s_idx, hc, 384:512],
                                 start=False, stop=False,
                                 tile_position=(0, 32 * hc))
            for hc in range(4):
                nc.tensor.matmul(P2[32 * hc:32 * hc + BC, :], SB,
                                 mRBt[:, s_idx, hc, 384:512],
                                 start=False, stop=True,
                                 tile_position=(0, 32 * hc))

            # elementwise tail in transposed space; sifo cols (i, g', f, o)
            sifo = work_pool.tile([128, 512], BF, name="sifo", tag="sifo")
            nc.scalar.activation(sifo[:, 0:384], P1[:], Sigmoid)
            nc.scalar.activation(sifo[:, 384:512], P2[:], Sigmoid)
            fT = ps_tr.tile([128, 128], BF, name="fT", tag="fT", bufs=1)
            nc.tensor.transpose(fT[:], sifo[:, 256:384], i128t[:])
            t2 = work_pool.tile([128, 128], BF, name="t2", tag="t2")
            nc.vector.scalar_tensor_tensor(t2[:], sifo[:, 128:256], 0.5,
                                           sifo[:, 0:128],
                                           ALU.subtract, ALU.mult)
            t2T = ps_tr.tile([128, 128], BF, name="t2T", tag="t2T", bufs=1)
            nc.tensor.transpose(t2T[:], t2[:], i128t[:])
            oT = ps_tr.tile([128, 128], BF, name="oT", tag="oT", bufs=1)
            nc.tensor.transpose(oT[:], sifo[:, 384:512], i128t[:])
            # HAM keep-warm filler: dummy rounds on the otherwise-idle PE while
            # the ACT/DVE chain runs; shares the P1 slot (WAR-serialized after
            # sigma1's read, results never consumed before the next overwrite)
            dmy = ps_g.tile([128, 384], F32, name="P1d", tag="P1", bufs=1)
            for r in range(5):
                for hc in range(4):
                    nc.tensor.matmul(dmy[32 * hc:32 * hc + BC, :], hTT[:, 0:16],
                                     wGt[:, r % 4, hc, 0:384],
                                     start=(r == 0), stop=(r == 4),
                                     tile_position=(0, 32 * hc))
            u = work_pool.tile([128, 128], BF, name="u", tag="u")
            nc.vector.tensor_mul(u[:], fT[:], CT[:])
            nc.vector.scalar_tensor_tensor(CT[:], t2T[:], 2.0, u[:],
                                           ALU.mult, ALU.add)
            tch = work_pool.tile([128, 128], BF, name="tch", tag="tch")
            nc.scalar.activation(tch[:], CT[:], Tanh)
            hTT = state_pool.tile([128, 128], BF, name="hTT", tag="hTT")
            nc.vector.tensor_mul(hTT[:, 0:64], oT[:, 0:64], tch[:, 0:64])
            nc.vector.tensor_mul(hTT[:, 64:128], oT[:, 64:128], tch[:, 64:128])

            # head: lf0 (cols 0:16) + uv pre-act (cols 16:32), both partition 0
            phd = ps_hd.tile([1, 32], F32, name="phd", tag="phd")
            for kk in range(4):
                nc.tensor.matmul(phd[0:1, 0:16], hwTt[:, kk, 0:1],
                                 hTT[:, 32 * kk:32 * kk + 16],
                                 start=(kk == 0), stop=(kk == 3))
            for kk in range(4):
                nc.tensor.matmul(phd[0:1, 16:32], hwTt[:, kk, 1:2],
                                 hTT[:, 32 * kk:32 * kk + 16],
                                 start=(kk == 0), stop=(kk == 3))
            o0 = 16 * (t + 1)
            nc.vector.tensor_scalar_add(outA[0:1, o0:o0 + 16], phd[0:1, 0:16],
                                        hb2t[0:1, 0:1])
            nc.scalar.activation(outB[0:1, o0:o0 + 16], phd[0:1, 16:32], Sigmoid,
                                 bias=hb2t[0:1, 1:2])

        OFl = lstm_pool.tile([1, T, 16], F32)
        nc.scalar.copy(OFl[:], outA[0:1, 16:16 * (T + 1)].rearrange("p (t b) -> p t b", t=T))
        OFu = lstm_pool.tile([1, T, 16], F32)
        nc.scalar.copy(OFu[:], outB[0:1, 16:16 * (T + 1)].rearrange("p (t b) -> p t b", t=T))
        nc.sync.dma_start(d_out.ap()[0:1], OFl[:])
        nc.sync.dma_start(d_out.ap()[1:2], OFu[:])

    nc.compile()
    return nc


# --------------------------------------------------------------------------
# entry point
# --------------------------------------------------------------------------

def _in_maps(P, T):
    shared = {k: P[k] for k in ["w0T", "cb0", "w1p", "cb1", "w2T", "cb2", "w3T", "cb3",
                                "w4R", "cb4", "pwT", "pb", "wG", "mRA", "mRB", "hwT",
                                "hb2", "i128"]}
    oinitA = np.zeros((2, 16 * (T + 1)), BF_NP)
    oinitA[1, :] = 1.0
    shared["oinitA"] = oinitA
    shared["oinitB"] = np.zeros((1, 16 * (T + 1)), BF_NP)
    in_maps = []
    for c in range(NCORES):
        m = dict(shared)
        m["t0"] = np.ascontiguousarray(P["t0_full"][:, BC * c:BC * c + BC, :])
        in_maps.append(m)
    return in_maps


def kernel(**inputs):
    T = int(np.asarray(inputs["num_steps"]))
    if T not in _CACHE:
        _CACHE[T] = _build(T)
    nc = _CACHE[T]
    P = _prep(inputs)
    in_maps = _in_maps(P, T)
    res = run_bass_kernel_spmd(nc, in_maps, list(range(NCORES)))
    out = np.empty((128, T, 2), np.float32)
    for c in range(NCORES):
        out[BC * c:BC * c + BC] = res.results[c]["out"].transpose(2, 1, 0)
    return out


# revision 17
# speedup vs baseline: 1.1913x; 1.1913x over previous
"""Trainium2 Bass kernel for nn_F0Predictor (conv stack + LSTM decode), 8-core data-parallel.

Contract: kernel(**inputs) takes the FULL unsharded inputs (as produced by
setup_inputs()) and returns the full [128, num_steps, 2] float32 output.
Internally: batch is sharded 8 ways (16 per NeuronCore), weights replicated,
compute in bf16 with fp32 PSUM accumulation. No collectives.

LSTM step design (v2):
  - gates psum P[128,512]: partition 32*hc+b, col 128*g+u, gate order (i,f,o,g)
  - g-gate rows pre-scaled by 2 host-side so one Sigmoid over all 512 cols
    gives sigma(i,f,o) and sigma(2g) (tanh(g) = 2*sigma(2g)-1, folded into the
    DVE chain via scalar_tensor_tensor)
  - x_t (rank-2 + bias) folded into one aux matmul with lhsT rows
    (lf0, sigma(uv), 1)
  - all elementwise state in bf16 -> DVE 2x mode
"""
import numpy as np
import ml_dtypes

import concourse.bass as bass
import concourse.tile as tile
from concourse import bacc, mybir
from concourse.bass_utils import run_bass_kernel_spmd

BF = mybir.dt.bfloat16
F32 = mybir.dt.float32
BF_NP = ml_dtypes.bfloat16
F8 = mybir.dt.float8e4
F8_NP = ml_dtypes.float8_e4m3

NCORES = 8
BC = 16          # batch per core
# torch LSTM row offsets for gate order (i, g, f, o)
TGOFF = [0, 1024, 512, 1536]
Sigmoid = mybir.ActivationFunctionType.Sigmoid
Tanh = mybir.ActivationFunctionType.Tanh
Relu = mybir.ActivationFunctionType.Relu
ALU = mybir.AluOpType

_CACHE = {}


# --------------------------------------------------------------------------
# host-side prep (numpy): weight layout transforms, batch sharding
# --------------------------------------------------------------------------

def _prep(inp):
    f32 = np.float32
    P = {}
    x = np.asarray(inp["x"], f32).reshape(128, 8192)
    x_pad = np.zeros((128, 8224), f32)
    x_pad[:, 16:8208] = x
    T0 = np.stack([x_pad[:, k + 1: k + 1 + 8192: 4] for k in range(31)], 0)  # [31,128,2048]
    P["t0_full"] = T0.astype(BF_NP)

    w0 = np.asarray(inp["cw0"], f32)
    P["w0T"] = w0[:, 0, :].T.astype(BF_NP).copy()                 # [31, 64]
    P["cb0"] = np.asarray(inp["cb0"], f32).reshape(64, 1).copy()

    w1 = np.asarray(inp["cw1"], f32)
    w1p = np.zeros((128, 16, 128), f32)               # [r, kp, co]
    for k in range(16):
        w1p[0:64, k, :] = w1[:, :, 2 * k].T
        if 2 * k + 1 <= 30:
            w1p[64:128, k, :] = w1[:, :, 2 * k + 1].T
    P["w1p"] = w1p.astype(BF_NP)
    P["cb1"] = np.asarray(inp["cb1"], f32).reshape(128, 1).copy()

    w2 = np.asarray(inp["cw2"], f32)
    w2T = np.zeros((128, 31, 2, 128), f32)                        # [r, k, cc, co]
    for k in range(31):
        for cc in range(2):
            w2T[:, k, cc, :] = w2[128 * cc:128 * cc + 128, :, k].T
    P["w2T"] = w2T.astype(BF_NP)
    P["cb2"] = np.ascontiguousarray(np.asarray(inp["cb2"], f32).reshape(2, 128).T)

    w3 = np.asarray(inp["cw3"], f32)
    w3T = np.zeros((128, 31, 2, 4, 128), f32)                     # [r, k, ci, cc, co]
    for k in range(31):
        for ci in range(2):
            for cc in range(4):
                w3T[:, k, ci, cc, :] = w3[128 * cc:128 * cc + 128, 128 * ci:128 * ci + 128, k].T
    P["w3T"] = w3T.astype(BF_NP)
    P["cb3"] = np.ascontiguousarray(np.asarray(inp["cb3"], f32).reshape(4, 128).T)

    w4 = np.asarray(inp["cw4"], f32)
    w4R = np.zeros((31, 4, 128, 1024), f32)                       # [k, ci, r, co]
    for k in range(31):
        for ci in range(4):
            w4R[k, ci] = w4[:, 128 * ci:128 * ci + 128, k].T
    P["w4R"] = (w4R * 16.0).astype(F8_NP)
    P["cb4"] = (np.asarray(inp["cb4"], f32).reshape(1, 1024) * 16.0).astype(BF_NP)

    phw = np.asarray(inp["ph_w"], f32)
    pcw = np.asarray(inp["pc_w"], f32)
    pwT = np.zeros((64, 128, 2, 4, 128), f32)                     # [kk, r, s, hc, uu]
    for kk in range(64):
        for hc in range(4):
            pwT[kk, :, 0, hc, :] = phw[128 * hc:128 * hc + 128, 128 * kk:128 * kk + 128].T
            pwT[kk, :, 1, hc, :] = pcw[128 * hc:128 * hc + 128, 128 * kk:128 * kk + 128].T
    P["pwT"] = (pwT * 16.0).astype(F8_NP)
    pb = np.zeros((1, 2, 4, 128), f32)
    pb[0, 0] = np.asarray(inp["ph_b"], f32).reshape(4, 128)
    pb[0, 1] = np.asarray(inp["pc_b"], f32).reshape(4, 128)
    P["pb"] = (pb * 16.0).astype(BF_NP)

    # ---- LSTM weights, v2 layout -------------------------------------
    whh = np.asarray(inp["w_hh"], f32)
    # wG[r, kk, hc, 128*g + u] = whh[TGOFF[g] + 128*hc + u, 128*kk + r]
    # g-gate block (g==3) pre-scaled by 2 for the sigmoid-only trick.
    wG = np.zeros((128, 4, 4, 512), f32)
    for kk in range(4):
        for hc in range(4):
            for g in range(4):
                blk = whh[TGOFF[g] + 128 * hc: TGOFF[g] + 128 * hc + 128,
                          128 * kk:128 * kk + 128].T   # [r, u]
                wG[:, kk, hc, 128 * g:128 * g + 128] = blk * (2.0 if g == 1 else 1.0)
    P["wG"] = wG.astype(BF_NP)

    wih = np.asarray(inp["w_ih"], f32)
    embw = np.asarray(inp["emb_w"], f32)
    M = wih @ embw                                # [2048, 2]
    const0 = np.asarray(inp["b_ih"], f32) + np.asarray(inp["b_hh"], f32)
    consts = const0 + wih @ np.asarray(inp["emb_b"], f32)
    # mRA rows (lf0 coeff, const); mRB row (uv coeff); [row, s, hc, 128*g + u]
    mRA = np.zeros((2, 2, 4, 512), f32)
    mRB = np.zeros((1, 2, 4, 512), f32)
    for hc in range(4):
        for g in range(4):
            sl = slice(TGOFF[g] + 128 * hc, TGOFF[g] + 128 * hc + 128)
            sc = 2.0 if g == 1 else 1.0
            c = slice(128 * g, 128 * g + 128)
            mRA[0, 1, hc, c] = M[sl, 0] * sc
            mRA[1, 0, hc, c] = const0[sl] * sc
            mRA[1, 1, hc, c] = consts[sl] * sc
            mRB[0, 1, hc, c] = M[sl, 1] * sc
    P["mRA"] = mRA.astype(BF_NP)
    P["mRB"] = mRB.astype(BF_NP)

    hwT = np.zeros((128, 4, 2), f32)
    for kk in range(4):
        hwT[:, kk, 0] = np.asarray(inp["lf0_w"], f32)[0, 128 * kk:128 * kk + 128]
        hwT[:, kk, 1] = np.asarray(inp["uv_w"], f32)[0, 128 * kk:128 * kk + 128]
    P["hwT"] = hwT.astype(BF_NP)
    P["hb2"] = np.array([[np.asarray(inp["lf0_b"], f32).reshape(-1)[0],
                          np.asarray(inp["uv_b"], f32).reshape(-1)[0]]], f32)
    P["i128"] = np.eye(128, dtype=BF_NP)
    return P


# --------------------------------------------------------------------------
# device program
# --------------------------------------------------------------------------

def _build(T):
    nc = bacc.Bacc("TRN2", target_bir_lowering=False, debug=False, num_devices=NCORES)

    d_t0 = nc.dram_tensor("t0", [31, BC, 2048], BF, kind="ExternalInput")
    d_w0 = nc.dram_tensor("w0T", [31, 64], BF, kind="ExternalInput")
    d_cb0 = nc.dram_tensor("cb0", [64, 1], F32, kind="ExternalInput")
    d_w1 = nc.dram_tensor("w1p", [128, 16, 128], BF, kind="ExternalInput")
    d_cb1 = nc.dram_tensor("cb1", [128, 1], F32, kind="ExternalInput")
    d_w2 = nc.dram_tensor("w2T", [128, 31, 2, 128], BF, kind="ExternalInput")
    d_cb2 = nc.dram_tensor("cb2", [128, 2], F32, kind="ExternalInput")
    d_w3 = nc.dram_tensor("w3T", [128, 31, 2, 4, 128], BF, kind="ExternalInput")
    d_cb3 = nc.dram_tensor("cb3", [128, 4], F32, kind="ExternalInput")
    d_w4 = nc.dram_tensor("w4R", [31, 4, 128, 1024], F8, kind="ExternalInput")
    d_cb4 = nc.dram_tensor("cb4", [1, 1024], BF, kind="ExternalInput")
    d_pw = nc.dram_tensor("pwT", [64, 128, 2, 4, 128], F8, kind="ExternalInput")
    d_pb = nc.dram_tensor("pb", [1, 2, 4, 128], BF, kind="ExternalInput")
    d_wG = nc.dram_tensor("wG", [128, 4, 4, 512], BF, kind="ExternalInput")
    d_mRA = nc.dram_tensor("mRA", [2, 2, 4, 512], BF, kind="ExternalInput")
    d_mRB = nc.dram_tensor("mRB", [1, 2, 4, 512], BF, kind="ExternalInput")
    d_oinitA = nc.dram_tensor("oinitA", [2, 16 * (T + 1)], BF, kind="ExternalInput")
    d_oinitB = nc.dram_tensor("oinitB", [1, 16 * (T + 1)], BF, kind="ExternalInput")
    d_hwT = nc.dram_tensor("hwT", [128, 4, 2], BF, kind="ExternalInput")
    d_hb2 = nc.dram_tensor("hb2", [1, 2], F32, kind="ExternalInput")
    d_i128 = nc.dram_tensor("i128", [128, 128], BF, kind="ExternalInput")
    d_out = nc.dram_tensor("out", [2, T, 16], F32, kind="ExternalOutput")
    d_warm = nc.dram_tensor("warm", [1, 16], F32, kind="ExternalOutput")
    d_warm2 = nc.dram_tensor("warm2", [1, 16], F32, kind="ExternalOutput")

    from contextlib import ExitStack
    with tile.TileContext(nc) as tc, ExitStack() as top:
        const_pool = top.enter_context(tc.tile_pool(name="const", bufs=1))
        i128t = const_pool.tile([128, 128], BF)
        nc.sync.dma_start(i128t[:], d_i128.ap())
        hb2t = const_pool.tile([1, 2], F32)
        nc.sync.dma_start(hb2t[:], d_hb2.ap())

        # persistent activations for the conv chain
        act1_pool = top.enter_context(tc.tile_pool(name="act1", bufs=1))
        act2_pool = top.enter_context(tc.tile_pool(name="act2", bufs=1))
        act3_pool = top.enter_context(tc.tile_pool(name="act3", bufs=1))
        out4_pool = top.enter_context(tc.tile_pool(name="out4", bufs=1))

        act1 = act1_pool.tile([128, BC, 543], BF)
        nc.gpsimd.memset(act1[:], 0.0)
        act2 = [act2_pool.tile([128, BC, 159], BF, name=f"act2_{i}", tag=f"act2_{i}") for i in range(2)]
        for t_ in act2:
            nc.gpsimd.memset(t_[:], 0.0)
        act3 = [act3_pool.tile([128, BC, 63], BF, name=f"act3_{i}", tag=f"act3_{i}") for i in range(4)]
        for t_ in act3:
            nc.gpsimd.memset(t_[:], 0.0)
        out4T = out4_pool.tile([128, 1024], BF)

        # prefetch L2 weights early (DMA overlaps L0/L1 compute)
        es_w2 = ExitStack()
        p2p = es_w2.enter_context(tc.tile_pool(name="p2", bufs=1))
        w2t = p2p.tile([128, 31, 2, 128], BF)
        nc.sync.dma_start(w2t[:], d_w2.ap())
        cb2t = p2p.tile([128, 2], F32)
        nc.sync.dma_start(cb2t[:], d_cb2.ap())

        # ---------------- L0 + L1 (own pools, freed after) ----------------
        with ExitStack() as es01:
            p01 = es01.enter_context(tc.tile_pool(name="p01", bufs=1))
            ps01 = es01.enter_context(tc.tile_pool(name="ps01", bufs=2, space="PSUM"))
            t0t = p01.tile([31, BC, 2048], BF)
            nc.sync.dma_start(t0t[:], d_t0.ap())
            w0t = p01.tile([31, 64], BF)
            nc.sync.dma_start(w0t[:], d_w0.ap())
            cb0t = p01.tile([64, 1], F32)
            nc.sync.dma_start(cb0t[:], d_cb0.ap())
            act0 = p01.tile([128, BC, 2079], BF)
            nc.gpsimd.memset(act0[:], 0.0)

            for bg in range(4):
                for lc in range(16):
                    p = ps01.tile([64, 4, 128], F32, name="l0ps", tag="l0ps")
                    nc.tensor.matmul(p[:], w0t[:],
                                     t0t[:, 4 * bg:4 * bg + 4, 128 * lc:128 * lc + 128],
                                     start=True, stop=True)
                    nc.scalar.activation(
                        act0[0:64, 4 * bg:4 * bg + 4, 15 + 128 * lc:15 + 128 * lc + 128],
                        p[:], Relu, bias=cb0t[:])
                # duplicate shifted by +1 element into partitions 64..127,
                # chunked per bg so the copy overlaps the next bg's matmuls
                nc.sync.dma_start(act0[64:128, 4 * bg:4 * bg + 4, 0:2078],
                                  act0[0:64, 4 * bg:4 * bg + 4, 1:2079])

            w1t = p01.tile([128, 16, 128], BF)
            nc.sync.dma_start(w1t[:], d_w1.ap())
            cb1t = p01.tile([128, 1], F32)
            nc.sync.dma_start(cb1t[:], d_cb1.ap())

            for bg in range(4):
                for lc in range(4):
                    p1 = ps01.tile([128, 4, 128], F32, name="l1ps", tag="l1ps", bufs=4)
                    for kp in range(16):
                        j0 = 2 * kp + 512 * lc
                        rhs = act0[:, 4 * bg:4 * bg + 4, j0: j0 + 512: 4]
                        nc.tensor.matmul(p1[:], w1t[:, kp, :], rhs,
                                         start=(kp == 0), stop=(kp == 15))
                    nc.scalar.activation(
                        act1[:, 4 * bg:4 * bg + 4, 15 + 128 * lc:15 + 128 * lc + 128],
                        p1[:], Relu, bias=cb1t[:])

        # prefetch L3 weights (DMA overlaps L2 compute)
        es_w3 = ExitStack()
        p3p = es_w3.enter_context(tc.tile_pool(name="p3", bufs=1))
        w3t = p3p.tile([128, 31, 2, 4, 128], BF)
        nc.sync.dma_start(w3t[:], d_w3.ap())
        cb3t = p3p.tile([128, 4], F32)
        nc.sync.dma_start(cb3t[:], d_cb3.ap())

        # ---------------- L2 ----------------
        with ExitStack() as es2:
            ps2 = es2.enter_context(tc.tile_pool(name="ps2", bufs=1, space="PSUM"))
            for cc in range(2):
                p2 = [ps2.tile([128, 4, 128], F32, name=f"l2ps_{bg}", tag=f"l2ps_{bg}") for bg in range(4)]
                for k in range(31):
                    for bg in range(4):
                        rhs = act1[:, 4 * bg:4 * bg + 4, k: k + 512: 4]
                        nc.tensor.matmul(p2[bg][:], w2t[:, k, cc, :], rhs,
                                         start=(k == 0), stop=(k == 30))
                for bg in range(4):
                    nc.scalar.activation(act2[cc][:, 4 * bg:4 * bg + 4, 15:143],
                                         p2[bg][:], Relu, bias=cb2t[:, cc:cc+1])

        # ---------------- L3 ----------------
        with ExitStack() as es3:
            ps3 = es3.enter_context(tc.tile_pool(name="ps3", bufs=2, space="PSUM"))
            for cc in range(4):
                p3 = ps3.tile([128, BC, 32], F32, name="l3ps", tag="l3ps")
                n = 0
                for ci in range(2):
                    for k in range(31):
                        rhs = act2[ci][:, :, k:k + 128:4]
                        nc.tensor.matmul(p3[:], w3t[:, k, ci, cc, :], rhs,
                                         start=(n == 0), stop=(n == 61))
                        n += 1
                nc.scalar.activation(act3[cc][:, :, 15:47], p3[:], Relu, bias=cb3t[:, cc:cc+1])
        es_w3.close()
        es_w2.close()

        # ---------------- L4 (weights moving) ----------------
        with ExitStack() as es4:
            p4p = es4.enter_context(tc.tile_pool(name="p4", bufs=8))
            p4c = es4.enter_context(tc.tile_pool(name="p4c", bufs=1))
            ps4 = es4.enter_context(tc.tile_pool(name="ps4", bufs=1, space="PSUM"))
            ones1 = p4c.tile([1, 128], BF)
            nc.gpsimd.memset(ones1[:], 1.0)
            cb4t = p4c.tile([1, 1024], BF)
            nc.sync.dma_start(cb4t[:], d_cb4.ap())
            PT = [ps4.tile([128, 512], F32, name=f"l4ps_{j}", tag=f"l4ps_{j}") for j in range(2)]
            for j in range(2):
                nc.tensor.matmul(PT[j][:], ones1[:, 0:128], cb4t[:, 512 * j:512 * j + 512],
                                 start=True, stop=False)
            for k in range(31):
                for ci in range(4):
                    w4c = p4p.tile([128, 1024], F8, name="w4c", tag="w4c", bufs=12)
                    nc.sync.dma_start(w4c[:], d_w4.ap()[k, ci])
                    imt = p4p.tile([128, 8, 16], F8, name="imt", tag="imt", bufs=4)
                    nc.vector.tensor_copy(
                        imt[:], act3[ci][:, :, k:k + 32:4].rearrange("p b l -> p l b"))
                    last = (k == 30 and ci == 3)
                    for j in range(2):
                        nc.tensor.matmul(PT[j][:], imt[:], w4c[:, 512 * j:512 * j + 512],
                                         start=False, stop=last)
            for j in range(2):
                nc.scalar.activation(out4T[:, 512 * j:512 * j + 512], PT[j][:], Relu,
                                     scale=1.0 / 16.0)

        # ---------------- transposes + projections ----------------
        lstm_pool = top.enter_context(tc.tile_pool(name="lstm", bufs=1))
        C = lstm_pool.tile([128, 128], BF)
        outA = lstm_pool.tile([2, 16 * (T + 1)], BF)   # rows (lf0, ones)
        outB = lstm_pool.tile([1, 16 * (T + 1)], BF)   # row sig(uv)
        nc.sync.dma_start(outA[:], d_oinitA.ap())
        nc.sync.dma_start(outB[:], d_oinitB.ap())

        state_pool = top.enter_context(tc.tile_pool(name="state", bufs=2))
        ps_tr = top.enter_context(tc.tile_pool(name="ps_tr", bufs=1, space="PSUM"))

        with ExitStack() as esp:
            ppw = esp.enter_context(tc.tile_pool(name="ppw", bufs=8))
            ppc = esp.enter_context(tc.tile_pool(name="ppc", bufs=1))
            psp = esp.enter_context(tc.tile_pool(name="psp", bufs=1, space="PSUM"))
            hfT = ppc.tile([128, 1024], F8)
            # transpose out4T[l*16+b, co] -> hfT[:, 16*kk+b] (kk = l*8 + c8),
            # two l-values per [32,128] transpose (base partitions 0/32/64/96)
            for q in range(4):
                ptile = ps_tr.tile([128, 8, 2, 16], BF, name="trp2", tag="trp")
                for c8 in range(8):
                    nc.tensor.transpose(
                        ptile[:, c8, :, :],
                        out4T[32 * q:32 * q + 32, 128 * c8:128 * c8 + 128],
                        i128t[32 * q:32 * q + 32, 32 * q:32 * q + 32],
                        tile_position=(32 * q, 0))
                dst = hfT[:, 256 * q:256 * q + 256].rearrange(
                    "p (l cc b) -> p cc l b", l=2, cc=8, b=16)
                nc.scalar.copy(dst, ptile[:])

            onesb = ppc.tile([1, 16], BF)
            nc.gpsimd.memset(onesb[:], 1.0)
            pbt = ppc.tile([1, 2, 4, 128], BF)
            nc.sync.dma_start(pbt[:], d_pb.ap())
            psh = [psp.tile([128, 128], F32, name=f"psh_{s}", tag=f"psh_{s}") for s in range(2)]
            for s in range(2):
                for hc in range(4):
                    nc.tensor.matmul(psh[s][32 * hc:32 * hc + BC, :], onesb[:],
                                     pbt[:, s, hc, :], start=True, stop=False,
                                     tile_position=(0, 32 * hc))
            for kk in range(64):
                pwc = ppw.tile([128, 2, 4, 128], F8, name="pwc", tag="pwc", bufs=16)
                nc.sync.dma_start(pwc[:], d_pw.ap()[kk])
                last = (kk == 63)
                for s in range(2):
                    for hc in range(4):
                        nc.tensor.matmul(psh[s][32 * hc:32 * hc + BC, :],
                                         hfT[:, 16 * kk:16 * kk + 16],
                                         pwc[:, s, hc, :], start=False, stop=last,
                                         tile_position=(0, 32 * hc))
            Hb0 = state_pool.tile([128, 128], BF, name="Hb", tag="Hb")
            nc.scalar.mul(Hb0[:], psh[0][:], 1.0 / 16.0)
            nc.scalar.mul(C[:], psh[1][:], 1.0 / 16.0)

        # ---------------- LSTM ----------------
        wGt = lstm_pool.tile([128, 4, 4, 512], BF)
        nc.sync.dma_start(wGt[:], d_wG.ap())
        mRAt = lstm_pool.tile([2, 2, 4, 512], BF)
        nc.sync.dma_start(mRAt[:], d_mRA.ap())
        mRBt = lstm_pool.tile([1, 2, 4, 512], BF)
        nc.sync.dma_start(mRBt[:], d_mRB.ap())
        hwTt = lstm_pool.tile([128, 4, 2], BF)
        nc.sync.dma_start(hwTt[:], d_hwT.ap())

        ps_g = top.enter_context(tc.tile_pool(name="ps_g", bufs=1, space="PSUM"))
        ps_hd = top.enter_context(tc.tile_pool(name="ps_hd", bufs=1, space="PSUM"))
        work_pool = top.enter_context(tc.tile_pool(name="work", bufs=2))

        def trans_h(hb):
            pt = ps_tr.tile([128, 128], BF, name="trp", tag="trp")
            nc.tensor.transpose(pt[:], hb[:], i128t[:])
            hTT = state_pool.tile([128, 128], BF, name="hTT", tag="hTT")
            nc.scalar.copy(hTT[:], pt[:])
            return hTT

        hTT = trans_h(Hb0)
        # HAM warm-up: >3.4us of dense matmuls so the LSTM runs at 2.4 GHz.
        # Output written to a junk DRAM tensor so the burst is not DCE'd.
        wu = ps_g.tile([128, 384], F32, name="P1", tag="P1", bufs=1)
        for r in range(16):
            for hc in range(4):
                nc.tensor.matmul(wu[32 * hc:32 * hc + BC, :], hTT[:, 0:16],
                                 wGt[:, r % 4, hc, 0:384],
                                 start=(r == 0), stop=(r == 15),
                                 tile_position=(0, 32 * hc))
        wscr = work_pool.tile([1, 16], F32, name="wscr", tag="wscr")
        nc.vector.tensor_copy(wscr[:], wu[0:1, 0:16])
        nc.sync.dma_start(d_warm.ap(), wscr[:])
        dmyt = ps_g.tile([128, 384], F32, name="Pd", tag="Pd", bufs=1)
        # transpose initial C into CT (cell state kept in transposed layout)
        ptc = ps_tr.tile([128, 128], BF, name="trp0", tag="trp")
        nc.tensor.transpose(ptc[:], C[:], i128t[:])
        CT = lstm_pool.tile([128, 128], BF)
        nc.vector.tensor_copy(CT[:], ptc[:])

        for t in range(T):
            s_idx = 0 if t == 0 else 1
            SA = outA[:, 16 * t:16 * t + 16]
            SB = outB[:, 16 * t:16 * t + 16]
            # bank 1: (i, g', f) gate columns — finishes early so the whole
            # sigmoid/DVE chain overlaps bank 2's (o-gate) streams
            P1 = ps_g.tile([128, 384], F32, name="P1", tag="P1", bufs=1)
            for kk in range(4):
                for hc in range(4):
                    nc.tensor.matmul(P1[32 * hc:32 * hc + BC, :],
                                     hTT[:, 32 * kk:32 * kk + 16],
                                     wGt[:, kk, hc, 0:384],
                                     start=(kk == 0), stop=False,
                                     tile_position=(0, 32 * hc))
            for hc in range(4):
                nc.tensor.matmul(P1[32 * hc:32 * hc + BC, :], SA,
                                 mRAt[:, s_idx, hc, 0:384],
                                 start=False, stop=False,
                                 tile_position=(0, 32 * hc))
            for hc in range(4):
                nc.tensor.matmul(P1[32 * hc:32 * hc + BC, :], SB,
                                 mRBt[:, s_idx, hc, 0:384],
                                 start=False, stop=True,
                                 tile_position=(0, 32 * hc))
            # bank 2: (o)
            P2 = ps_g.tile([128, 128], F32, name="P2", tag="P2", bufs=1)
            for kk in range(4):
                for hc in range(4):
                    nc.tensor.matmul(P2[32 * hc:32 * hc + BC, :],
                                     hTT[:, 32 * kk:32 * kk + 16],
                                     wGt[:, kk, hc, 384:512],
                                     start=(kk == 0), stop=False,
                                     tile_position=(0, 32 * hc))
            for hc in range(4):
                nc.tensor.matmul(P2[32 * hc:32 * hc + BC, :], SA,
                                 mRAt[:, s_idx, hc, 384:512],
                                 start=False, stop=False,
                                 tile_position=(0, 32 * hc))
            for hc in range(4):
                nc.tensor.matmul(P2[32 * hc:32 * hc + BC, :], SB,
                                 mRBt[:, s_idx, hc, 384:512],
                                 start=False, stop=True,
                                 tile_position=(0, 32 * hc))

            # elementwise tail in transposed space; sifo cols (i, g', f, o)
            sifo = work_pool.tile([128, 512], BF, name="sifo", tag="sifo")
            nc.scalar.activation(sifo[:, 0:384], P1[:], Sigmoid)
            nc.scalar.activation(sifo[:, 384:512], P2[:], Sigmoid)
            fT = ps_tr.tile([128, 128], BF, name="fT", tag="fT", bufs=1)
            nc.tensor.transpose(fT[:], sifo[:, 256:384], i128t[:])
            t2 = work_pool.tile([128, 128], BF, name="t2", tag="t2")
            nc.vector.scalar_tensor_tensor(t2[:], sifo[:, 128:256], 0.5,
                                           sifo[:, 0:128],
                                           ALU.subtract, ALU.mult)
            t2T = ps_tr.tile([128, 128], BF, name="t2T", tag="t2T", bufs=1)
            nc.tensor.transpose(t2T[:], t2[:], i128t[:])
            oT = ps_tr.tile([128, 128], BF, name="oT", tag="oT", bufs=1)
            nc.tensor.transpose(oT[:], sifo[:, 384:512], i128t[:])
            # HAM keep-warm filler: dummy rounds on the otherwise-idle PE while
            # the ACT/DVE chain runs (dedicated psum bank, read once after the
            # loop so it is not DCE'd)
            for r in range(5):
                for hc in range(4):
                    nc.tensor.matmul(dmyt[32 * hc:32 * hc + BC, :], hTT[:, 0:16],
                                     wGt[:, r % 4, hc, 0:384],
                                     start=(r == 0), stop=(r == 4),
                                     tile_position=(0, 32 * hc))
            u = work_pool.tile([128, 128], BF, name="u", tag="u")
            nc.vector.tensor_mul(u[:], fT[:], CT[:])
            nc.vector.scalar_tensor_tensor(CT[:], t2T[:], 2.0, u[:],
                                           ALU.mult, ALU.add)
            tch = work_pool.tile([128, 128], BF, name="tch", tag="tch")
            nc.scalar.activation(tch[:], CT[:], Tanh)
            hTT = state_pool.tile([128, 128], BF, name="hTT", tag="hTT")
            nc.vector.tensor_mul(hTT[:, 0:64], oT[:, 0:64], tch[:, 0:64])
            nc.vector.tensor_mul(hTT[:, 64:128], oT[:, 64:128], tch[:, 64:128])

            # head: lf0 (cols 0:16) + uv pre-act (cols 16:32), both partition 0
            phd = ps_hd.tile([1, 32], F32, name="phd", tag="phd")
            for kk in range(4):
                nc.tensor.matmul(phd[0:1, 0:16], hwTt[:, kk, 0:1],
                                 hTT[:, 32 * kk:32 * kk + 16],
                                 start=(kk == 0), stop=(kk == 3))
            for kk in range(4):
                nc.tensor.matmul(phd[0:1, 16:32], hwTt[:, kk, 1:2],
                                 hTT[:, 32 * kk:32 * kk + 16],
                                 start=(kk == 0), stop=(kk == 3))
            o0 = 16 * (t + 1)
            nc.vector.tensor_scalar_add(outA[0:1, o0:o0 + 16], phd[0:1, 0:16],
                                        hb2t[0:1, 0:1])
            nc.scalar.activation(outB[0:1, o0:o0 + 16], phd[0:1, 16:32], Sigmoid,
                                 bias=hb2t[0:1, 1:2])

        wscr2 = work_pool.tile([1, 16], F32, name="wscr2", tag="wscr")
        nc.vector.tensor_copy(wscr2[:], dmyt[0:1, 0:16])
        nc.sync.dma_start(d_warm2.ap(), wscr2[:])
        OFl = lstm_pool.tile([1, T, 16], F32)
        nc.scalar.copy(OFl[:], outA[0:1, 16:16 * (T + 1)].rearrange("p (t b) -> p t b", t=T))
        OFu = lstm_pool.tile([1, T, 16], F32)
        nc.scalar.copy(OFu[:], outB[0:1, 16:16 * (T + 1)].rearrange("p (t b) -> p t b", t=T))
        nc.sync.dma_start(d_out.ap()[0:1], OFl[:])
        nc.sync.dma_start(d_out.ap()[1:2], OFu[:])

    nc.compile()
    return nc


# --------------------------------------------------------------------------
# entry point
# --------------------------------------------------------------------------

def _in_maps(P, T):
    shared = {k: P[k] for k in ["w0T", "cb0", "w1p", "cb1", "w2T", "cb2", "w3T", "cb3",
                                "w4R", "cb4", "pwT", "pb", "wG", "mRA", "mRB", "hwT",
                                "hb2", "i128"]}
    oinitA = np.zeros((2, 16 * (T + 1)), BF_NP)
    oinitA[1, :] = 1.0
    shared["oinitA"] = oinitA
    shared["oinitB"] = np.zeros((1, 16 * (T + 1)), BF_NP)
    in_maps = []
    for c in range(NCORES):
        m = dict(shared)
        m["t0"] = np.ascontiguousarray(P["t0_full"][:, BC * c:BC * c + BC, :])
        in_maps.append(m)
    return in_maps


def kernel(**inputs):
    T = int(np.asarray(inputs["num_steps"]))
    if T not in _CACHE:
        _CACHE[T] = _build(T)
    nc = _CACHE[T]
    P = _prep(inputs)
    in_maps = _in_maps(P, T)
    res = run_bass_kernel_spmd(nc, in_maps, list(range(NCORES)))
    out = np.empty((128, T, 2), np.float32)
    for c in range(NCORES):
        out[BC * c:BC * c + BC] = res.results[c]["out"].transpose(2, 1, 0)
    return out


# revision 18
# speedup vs baseline: 1.1952x; 1.0032x over previous
"""Trainium2 Bass kernel for nn_F0Predictor (conv stack + LSTM decode), 8-core data-parallel.

Contract: kernel(**inputs) takes the FULL unsharded inputs (as produced by
setup_inputs()) and returns the full [128, num_steps, 2] float32 output.
Internally: batch is sharded 8 ways (16 per NeuronCore), weights replicated,
compute in bf16 with fp32 PSUM accumulation. No collectives.

LSTM step design (v2):
  - gates psum P[128,512]: partition 32*hc+b, col 128*g+u, gate order (i,f,o,g)
  - g-gate rows pre-scaled by 2 host-side so one Sigmoid over all 512 cols
    gives sigma(i,f,o) and sigma(2g) (tanh(g) = 2*sigma(2g)-1, folded into the
    DVE chain via scalar_tensor_tensor)
  - x_t (rank-2 + bias) folded into one aux matmul with lhsT rows
    (lf0, sigma(uv), 1)
  - all elementwise state in bf16 -> DVE 2x mode
"""
import numpy as np
import ml_dtypes

import concourse.bass as bass
import concourse.tile as tile
from concourse import bacc, mybir
from concourse.bass_utils import run_bass_kernel_spmd

BF = mybir.dt.bfloat16
F32 = mybir.dt.float32
BF_NP = ml_dtypes.bfloat16
F8 = mybir.dt.float8e4
F8_NP = ml_dtypes.float8_e4m3

NCORES = 8
BC = 16          # batch per core
# torch LSTM row offsets for gate order (i, g, f, o)
TGOFF = [0, 1024, 512, 1536]
Sigmoid = mybir.ActivationFunctionType.Sigmoid
Tanh = mybir.ActivationFunctionType.Tanh
Relu = mybir.ActivationFunctionType.Relu
ALU = mybir.AluOpType

_CACHE = {}


# --------------------------------------------------------------------------
# host-side prep (numpy): weight layout transforms, batch sharding
# --------------------------------------------------------------------------

def _prep(inp):
    f32 = np.float32
    P = {}
    x = np.asarray(inp["x"], f32).reshape(128, 8192)
    x_pad = np.zeros((128, 8224), f32)
    x_pad[:, 16:8208] = x
    T0 = np.stack([x_pad[:, k + 1: k + 1 + 8192: 4] for k in range(31)], 0)  # [31,128,2048]
    P["t0_full"] = T0.astype(BF_NP)

    w0 = np.asarray(inp["cw0"], f32)
    P["w0T"] = w0[:, 0, :].T.astype(BF_NP).copy()                 # [31, 64]
    P["cb0"] = np.asarray(inp["cb0"], f32).reshape(64, 1).copy()

    w1 = np.asarray(inp["cw1"], f32)
    w1p = np.zeros((128, 16, 128), f32)               # [r, kp, co]
    for k in range(16):
        w1p[0:64, k, :] = w1[:, :, 2 * k].T
        if 2 * k + 1 <= 30:
            w1p[64:128, k, :] = w1[:, :, 2 * k + 1].T
    P["w1p"] = w1p.astype(BF_NP)
    P["cb1"] = np.asarray(inp["cb1"], f32).reshape(128, 1).copy()

    w2 = np.asarray(inp["cw2"], f32)
    w2T = np.zeros((128, 31, 2, 128), f32)                        # [r, k, cc, co]
    for k in range(31):
        for cc in range(2):
            w2T[:, k, cc, :] = w2[128 * cc:128 * cc + 128, :, k].T
    P["w2T"] = w2T.astype(BF_NP)
    P["cb2"] = np.ascontiguousarray(np.asarray(inp["cb2"], f32).reshape(2, 128).T)

    w3 = np.asarray(inp["cw3"], f32)
    w3T = np.zeros((128, 31, 2, 4, 128), f32)                     # [r, k, ci, cc, co]
    for k in range(31):
        for ci in range(2):
            for cc in range(4):
                w3T[:, k, ci, cc, :] = w3[128 * cc:128 * cc + 128, 128 * ci:128 * ci + 128, k].T
    P["w3T"] = w3T.astype(BF_NP)
    P["cb3"] = np.ascontiguousarray(np.asarray(inp["cb3"], f32).reshape(4, 128).T)

    w4 = np.asarray(inp["cw4"], f32)
    w4R = np.zeros((31, 4, 128, 1024), f32)                       # [k, ci, r, co]
    for k in range(31):
        for ci in range(4):
            w4R[k, ci] = w4[:, 128 * ci:128 * ci + 128, k].T
    P["w4R"] = (w4R * 16.0).astype(F8_NP)
    P["cb4"] = (np.asarray(inp["cb4"], f32).reshape(1, 1024) * 16.0).astype(BF_NP)

    phw = np.asarray(inp["ph_w"], f32)
    pcw = np.asarray(inp["pc_w"], f32)
    pwT = np.zeros((64, 128, 2, 4, 128), f32)                     # [kk, r, s, hc, uu]
    for kk in range(64):
        for hc in range(4):
            pwT[kk, :, 0, hc, :] = phw[128 * hc:128 * hc + 128, 128 * kk:128 * kk + 128].T
            pwT[kk, :, 1, hc, :] = pcw[128 * hc:128 * hc + 128, 128 * kk:128 * kk + 128].T
    P["pwT"] = (pwT * 16.0).astype(F8_NP)
    pb = np.zeros((1, 2, 4, 128), f32)
    pb[0, 0] = np.asarray(inp["ph_b"], f32).reshape(4, 128)
    pb[0, 1] = np.asarray(inp["pc_b"], f32).reshape(4, 128)
    P["pb"] = (pb * 16.0).astype(BF_NP)

    # ---- LSTM weights, v2 layout -------------------------------------
    whh = np.asarray(inp["w_hh"], f32)
    # wG[r, kk, hc, 128*g + u] = whh[TGOFF[g] + 128*hc + u, 128*kk + r]
    # g-gate block (g==3) pre-scaled by 2 for the sigmoid-only trick.
    wG = np.zeros((128, 4, 4, 512), f32)
    for kk in range(4):
        for hc in range(4):
            for g in range(4):
                blk = whh[TGOFF[g] + 128 * hc: TGOFF[g] + 128 * hc + 128,
                          128 * kk:128 * kk + 128].T   # [r, u]
                wG[:, kk, hc, 128 * g:128 * g + 128] = blk * (2.0 if g == 1 else 1.0)
    P["wG"] = wG.astype(BF_NP)

    wih = np.asarray(inp["w_ih"], f32)
    embw = np.asarray(inp["emb_w"], f32)
    M = wih @ embw                                # [2048, 2]
    const0 = np.asarray(inp["b_ih"], f32) + np.asarray(inp["b_hh"], f32)
    consts = const0 + wih @ np.asarray(inp["emb_b"], f32)
    # mRA rows (lf0 coeff, const); mRB row (uv coeff); [row, s, hc, 128*g + u]
    mRA = np.zeros((2, 2, 4, 512), f32)
    mRB = np.zeros((1, 2, 4, 512), f32)
    for hc in range(4):
        for g in range(4):
            sl = slice(TGOFF[g] + 128 * hc, TGOFF[g] + 128 * hc + 128)
            sc = 2.0 if g == 1 else 1.0
            c = slice(128 * g, 128 * g + 128)
            mRA[0, 1, hc, c] = M[sl, 0] * sc
            mRA[1, 0, hc, c] = const0[sl] * sc
            mRA[1, 1, hc, c] = consts[sl] * sc
            mRB[0, 1, hc, c] = M[sl, 1] * sc
    P["mRA"] = mRA.astype(BF_NP)
    P["mRB"] = mRB.astype(BF_NP)

    hwT = np.zeros((128, 4, 2), f32)
    for kk in range(4):
        hwT[:, kk, 0] = np.asarray(inp["lf0_w"], f32)[0, 128 * kk:128 * kk + 128]
        hwT[:, kk, 1] = np.asarray(inp["uv_w"], f32)[0, 128 * kk:128 * kk + 128]
    P["hwT"] = hwT.astype(BF_NP)
    P["hb2"] = np.array([[np.asarray(inp["lf0_b"], f32).reshape(-1)[0],
                          np.asarray(inp["uv_b"], f32).reshape(-1)[0]]], f32)
    P["i128"] = np.eye(128, dtype=BF_NP)
    return P


# --------------------------------------------------------------------------
# device program
# --------------------------------------------------------------------------

def _build(T):
    nc = bacc.Bacc("TRN2", target_bir_lowering=False, debug=False, num_devices=NCORES)

    d_t0 = nc.dram_tensor("t0", [31, BC, 2048], BF, kind="ExternalInput")
    d_w0 = nc.dram_tensor("w0T", [31, 64], BF, kind="ExternalInput")
    d_cb0 = nc.dram_tensor("cb0", [64, 1], F32, kind="ExternalInput")
    d_w1 = nc.dram_tensor("w1p", [128, 16, 128], BF, kind="ExternalInput")
    d_cb1 = nc.dram_tensor("cb1", [128, 1], F32, kind="ExternalInput")
    d_w2 = nc.dram_tensor("w2T", [128, 31, 2, 128], BF, kind="ExternalInput")
    d_cb2 = nc.dram_tensor("cb2", [128, 2], F32, kind="ExternalInput")
    d_w3 = nc.dram_tensor("w3T", [128, 31, 2, 4, 128], BF, kind="ExternalInput")
    d_cb3 = nc.dram_tensor("cb3", [128, 4], F32, kind="ExternalInput")
    d_w4 = nc.dram_tensor("w4R", [31, 4, 128, 1024], F8, kind="ExternalInput")
    d_cb4 = nc.dram_tensor("cb4", [1, 1024], BF, kind="ExternalInput")
    d_pw = nc.dram_tensor("pwT", [64, 128, 2, 4, 128], F8, kind="ExternalInput")
    d_pb = nc.dram_tensor("pb", [1, 2, 4, 128], BF, kind="ExternalInput")
    d_wG = nc.dram_tensor("wG", [128, 4, 4, 512], BF, kind="ExternalInput")
    d_mRA = nc.dram_tensor("mRA", [2, 2, 4, 512], BF, kind="ExternalInput")
    d_mRB = nc.dram_tensor("mRB", [1, 2, 4, 512], BF, kind="ExternalInput")
    d_oinitA = nc.dram_tensor("oinitA", [2, 16 * (T + 1)], BF, kind="ExternalInput")
    d_oinitB = nc.dram_tensor("oinitB", [1, 16 * (T + 1)], BF, kind="ExternalInput")
    d_hwT = nc.dram_tensor("hwT", [128, 4, 2], BF, kind="ExternalInput")
    d_hb2 = nc.dram_tensor("hb2", [1, 2], F32, kind="ExternalInput")
    d_i128 = nc.dram_tensor("i128", [128, 128], BF, kind="ExternalInput")
    d_out = nc.dram_tensor("out", [2, T, 16], F32, kind="ExternalOutput")
    d_warm = nc.dram_tensor("warm", [1, 16], F32, kind="ExternalOutput")
    d_warm2 = nc.dram_tensor("warm2", [1, 16], F32, kind="ExternalOutput")
    d_warm0 = nc.dram_tensor("warm0", [1, 16], F32, kind="ExternalOutput")

    from contextlib import ExitStack
    with tile.TileContext(nc) as tc, ExitStack() as top:
        const_pool = top.enter_context(tc.tile_pool(name="const", bufs=1))
        i128t = const_pool.tile([128, 128], BF)
        nc.sync.dma_start(i128t[:], d_i128.ap())
        hb2t = const_pool.tile([1, 2], F32)
        nc.sync.dma_start(hb2t[:], d_hb2.ap())

        # persistent activations for the conv chain
        act1_pool = top.enter_context(tc.tile_pool(name="act1", bufs=1))
        act2_pool = top.enter_context(tc.tile_pool(name="act2", bufs=1))
        act3_pool = top.enter_context(tc.tile_pool(name="act3", bufs=1))
        out4_pool = top.enter_context(tc.tile_pool(name="out4", bufs=1))

        act1 = act1_pool.tile([128, BC, 543], BF)
        nc.gpsimd.memset(act1[:], 0.0)
        act2 = [act2_pool.tile([128, BC, 159], BF, name=f"act2_{i}", tag=f"act2_{i}") for i in range(2)]
        for t_ in act2:
            nc.gpsimd.memset(t_[:], 0.0)
        act3 = [act3_pool.tile([128, BC, 63], BF, name=f"act3_{i}", tag=f"act3_{i}") for i in range(4)]
        for t_ in act3:
            nc.gpsimd.memset(t_[:], 0.0)
        out4T = out4_pool.tile([128, 1024], BF)

        # prefetch L2 weights early (DMA overlaps L0/L1 compute)
        es_w2 = ExitStack()
        p2p = es_w2.enter_context(tc.tile_pool(name="p2", bufs=1))
        w2t = p2p.tile([128, 31, 2, 128], BF)
        nc.sync.dma_start(w2t[:], d_w2.ap())
        cb2t = p2p.tile([128, 2], F32)
        nc.sync.dma_start(cb2t[:], d_cb2.ap())

        # ---------------- L0 + L1 (own pools, freed after) ----------------
        with ExitStack() as es01:
            p01 = es01.enter_context(tc.tile_pool(name="p01", bufs=1))
            ps01 = es01.enter_context(tc.tile_pool(name="ps01", bufs=2, space="PSUM"))
            t0t = p01.tile([31, BC, 2048], BF)
            nc.sync.dma_start(t0t[:], d_t0.ap())
            w0t = p01.tile([31, 64], BF)
            nc.sync.dma_start(w0t[:], d_w0.ap())
            cb0t = p01.tile([64, 1], F32)
            nc.sync.dma_start(cb0t[:], d_cb0.ap())
            act0 = p01.tile([128, BC, 2079], BF)
            nc.gpsimd.memset(act0[:], 0.0)

            # HAM warm-up while the t0 DMA is in flight: dense dummy matmuls on
            # the identity tile so L0/L1 start at 2.4 GHz
            wu0 = ps01.tile([64, 128], F32, name="wu0", tag="wu0")
            for r in range(30):
                nc.tensor.matmul(wu0[:], i128t[:, 0:64], i128t[:],
                                 start=(r == 0), stop=(r == 29))
            wscr0 = p01.tile([1, 16], F32)
            nc.vector.tensor_copy(wscr0[:], wu0[0:1, 0:16])
            nc.sync.dma_start(d_warm0.ap(), wscr0[:])

            for bg in range(4):
                for lc in range(16):
                    p = ps01.tile([64, 4, 128], F32, name="l0ps", tag="l0ps")
                    nc.tensor.matmul(p[:], w0t[:],
                                     t0t[:, 4 * bg:4 * bg + 4, 128 * lc:128 * lc + 128],
                                     start=True, stop=True)
                    nc.scalar.activation(
                        act0[0:64, 4 * bg:4 * bg + 4, 15 + 128 * lc:15 + 128 * lc + 128],
                        p[:], Relu, bias=cb0t[:])
                # duplicate shifted by +1 element into partitions 64..127,
                # chunked per bg so the copy overlaps the next bg's matmuls
                nc.sync.dma_start(act0[64:128, 4 * bg:4 * bg + 4, 0:2078],
                                  act0[0:64, 4 * bg:4 * bg + 4, 1:2079])

            w1t = p01.tile([128, 16, 128], BF)
            nc.sync.dma_start(w1t[:], d_w1.ap())
            cb1t = p01.tile([128, 1], F32)
            nc.sync.dma_start(cb1t[:], d_cb1.ap())

            for bg in range(4):
                for lc in range(4):
                    p1 = ps01.tile([128, 4, 128], F32, name="l1ps", tag="l1ps", bufs=4)
                    for kp in range(16):
                        j0 = 2 * kp + 512 * lc
                        rhs = act0[:, 4 * bg:4 * bg + 4, j0: j0 + 512: 4]
                        nc.tensor.matmul(p1[:], w1t[:, kp, :], rhs,
                                         start=(kp == 0), stop=(kp == 15))
                    nc.scalar.activation(
                        act1[:, 4 * bg:4 * bg + 4, 15 + 128 * lc:15 + 128 * lc + 128],
                        p1[:], Relu, bias=cb1t[:])

        # prefetch L3 weights (DMA overlaps L2 compute)
        es_w3 = ExitStack()
        p3p = es_w3.enter_context(tc.tile_pool(name="p3", bufs=1))
        w3t = p3p.tile([128, 31, 2, 4, 128], BF)
        nc.sync.dma_start(w3t[:], d_w3.ap())
        cb3t = p3p.tile([128, 4], F32)
        nc.sync.dma_start(cb3t[:], d_cb3.ap())

        # ---------------- L2 ----------------
        with ExitStack() as es2:
            ps2 = es2.enter_context(tc.tile_pool(name="ps2", bufs=1, space="PSUM"))
            for cc in range(2):
                p2 = [ps2.tile([128, 4, 128], F32, name=f"l2ps_{bg}", tag=f"l2ps_{bg}") for bg in range(4)]
                for k in range(31):
                    for bg in range(4):
                        rhs = act1[:, 4 * bg:4 * bg + 4, k: k + 512: 4]
                        nc.tensor.matmul(p2[bg][:], w2t[:, k, cc, :], rhs,
                                         start=(k == 0), stop=(k == 30))
                for bg in range(4):
                    nc.scalar.activation(act2[cc][:, 4 * bg:4 * bg + 4, 15:143],
                                         p2[bg][:], Relu, bias=cb2t[:, cc:cc+1])

        # ---------------- L3 ----------------
        with ExitStack() as es3:
            ps3 = es3.enter_context(tc.tile_pool(name="ps3", bufs=2, space="PSUM"))
            for cc in range(4):
                p3 = ps3.tile([128, BC, 32], F32, name="l3ps", tag="l3ps")
                n = 0
                for ci in range(2):
                    for k in range(31):
                        rhs = act2[ci][:, :, k:k + 128:4]
                        nc.tensor.matmul(p3[:], w3t[:, k, ci, cc, :], rhs,
                                         start=(n == 0), stop=(n == 61))
                        n += 1
                nc.scalar.activation(act3[cc][:, :, 15:47], p3[:], Relu, bias=cb3t[:, cc:cc+1])
        es_w3.close()
        es_w2.close()

        # ---------------- L4 (weights moving) ----------------
        with ExitStack() as es4:
            p4p = es4.enter_context(tc.tile_pool(name="p4", bufs=8))
            p4c = es4.enter_context(tc.tile_pool(name="p4c", bufs=1))
            ps4 = es4.enter_context(tc.tile_pool(name="ps4", bufs=1, space="PSUM"))
            ones1 = p4c.tile([1, 128], BF)
            nc.gpsimd.memset(ones1[:], 1.0)
            cb4t = p4c.tile([1, 1024], BF)
            nc.sync.dma_start(cb4t[:], d_cb4.ap())
            PT = [ps4.tile([128, 512], F32, name=f"l4ps_{j}", tag=f"l4ps_{j}") for j in range(2)]
            for j in range(2):
                nc.tensor.matmul(PT[j][:], ones1[:, 0:128], cb4t[:, 512 * j:512 * j + 512],
                                 start=True, stop=False)
            for k in range(31):
                for ci in range(4):
                    w4c = p4p.tile([128, 1024], F8, name="w4c", tag="w4c", bufs=12)
                    nc.sync.dma_start(w4c[:], d_w4.ap()[k, ci])
                    imt = p4p.tile([128, 8, 16], F8, name="imt", tag="imt", bufs=4)
                    nc.vector.tensor_copy(
                        imt[:], act3[ci][:, :, k:k + 32:4].rearrange("p b l -> p l b"))
                    last = (k == 30 and ci == 3)
                    for j in range(2):
                        nc.tensor.matmul(PT[j][:], imt[:], w4c[:, 512 * j:512 * j + 512],
                                         start=False, stop=last)
            for j in range(2):
                nc.scalar.activation(out4T[:, 512 * j:512 * j + 512], PT[j][:], Relu,
                                     scale=1.0 / 16.0)

        # ---------------- transposes + projections ----------------
        lstm_pool = top.enter_context(tc.tile_pool(name="lstm", bufs=1))
        C = lstm_pool.tile([128, 128], BF)
        outA = lstm_pool.tile([2, 16 * (T + 1)], BF)   # rows (lf0, ones)
        outB = lstm_pool.tile([1, 16 * (T + 1)], BF)   # row sig(uv)
        nc.sync.dma_start(outA[:], d_oinitA.ap())
        nc.sync.dma_start(outB[:], d_oinitB.ap())

        state_pool = top.enter_context(tc.tile_pool(name="state", bufs=2))
        ps_tr = top.enter_context(tc.tile_pool(name="ps_tr", bufs=1, space="PSUM"))

        with ExitStack() as esp:
            ppw = esp.enter_context(tc.tile_pool(name="ppw", bufs=8))
            ppc = esp.enter_context(tc.tile_pool(name="ppc", bufs=1))
            psp = esp.enter_context(tc.tile_pool(name="psp", bufs=1, space="PSUM"))
            hfT = ppc.tile([128, 1024], F8)
            # transpose out4T[l*16+b, co] -> hfT[:, 16*kk+b] (kk = l*8 + c8),
            # two l-values per [32,128] transpose (base partitions 0/32/64/96)
            for q in range(4):
                ptile = ps_tr.tile([128, 8, 2, 16], BF, name="trp2", tag="trp")
                for c8 in range(8):
                    nc.tensor.transpose(
                        ptile[:, c8, :, :],
                        out4T[32 * q:32 * q + 32, 128 * c8:128 * c8 + 128],
                        i128t[32 * q:32 * q + 32, 32 * q:32 * q + 32],
                        tile_position=(32 * q, 0))
                dst = hfT[:, 256 * q:256 * q + 256].rearrange(
                    "p (l cc b) -> p cc l b", l=2, cc=8, b=16)
                nc.scalar.copy(dst, ptile[:])

            onesb = ppc.tile([1, 16], BF)
            nc.gpsimd.memset(onesb[:], 1.0)
            pbt = ppc.tile([1, 2, 4, 128], BF)
            nc.sync.dma_start(pbt[:], d_pb.ap())
            psh = [psp.tile([128, 128], F32, name=f"psh_{s}", tag=f"psh_{s}") for s in range(2)]
            for s in range(2):
                for hc in range(4):
                    nc.tensor.matmul(psh[s][32 * hc:32 * hc + BC, :], onesb[:],
                                     pbt[:, s, hc, :], start=True, stop=False,
                                     tile_position=(0, 32 * hc))
            for kk in range(64):
                pwc = ppw.tile([128, 2, 4, 128], F8, name="pwc", tag="pwc", bufs=16)
                nc.sync.dma_start(pwc[:], d_pw.ap()[kk])
                last = (kk == 63)
                for s in range(2):
                    for hc in range(4):
                        nc.tensor.matmul(psh[s][32 * hc:32 * hc + BC, :],
                                         hfT[:, 16 * kk:16 * kk + 16],
                                         pwc[:, s, hc, :], start=False, stop=last,
                                         tile_position=(0, 32 * hc))
            Hb0 = state_pool.tile([128, 128], BF, name="Hb", tag="Hb")
            nc.scalar.mul(Hb0[:], psh[0][:], 1.0 / 16.0)
            nc.scalar.mul(C[:], psh[1][:], 1.0 / 16.0)

        # ---------------- LSTM ----------------
        wGt = lstm_pool.tile([128, 4, 4, 512], BF)
        nc.sync.dma_start(wGt[:], d_wG.ap())
        mRAt = lstm_pool.tile([2, 2, 4, 512], BF)
        nc.sync.dma_start(mRAt[:], d_mRA.ap())
        mRBt = lstm_pool.tile([1, 2, 4, 512], BF)
        nc.sync.dma_start(mRBt[:], d_mRB.ap())
        hwTt = lstm_pool.tile([128, 4, 2], BF)
        nc.sync.dma_start(hwTt[:], d_hwT.ap())

        ps_g = top.enter_context(tc.tile_pool(name="ps_g", bufs=1, space="PSUM"))
        ps_hd = top.enter_context(tc.tile_pool(name="ps_hd", bufs=1, space="PSUM"))
        work_pool = top.enter_context(tc.tile_pool(name="work", bufs=2))

        def trans_h(hb):
            pt = ps_tr.tile([128, 128], BF, name="trp", tag="trp")
            nc.tensor.transpose(pt[:], hb[:], i128t[:])
            hTT = state_pool.tile([128, 128], BF, name="hTT", tag="hTT")
            nc.scalar.copy(hTT[:], pt[:])
            return hTT

        hTT = trans_h(Hb0)
        # HAM warm-up: >3.4us of dense matmuls so the LSTM runs at 2.4 GHz.
        # Output written to a junk DRAM tensor so the burst is not DCE'd.
        wu = ps_g.tile([128, 384], F32, name="P1", tag="P1", bufs=1)
        for r in range(16):
            for hc in range(4):
                nc.tensor.matmul(wu[32 * hc:32 * hc + BC, :], hTT[:, 0:16],
                                 wGt[:, r % 4, hc, 0:384],
                                 start=(r == 0), stop=(r == 15),
                                 tile_position=(0, 32 * hc))
        wscr = work_pool.tile([1, 16], F32, name="wscr", tag="wscr")
        nc.vector.tensor_copy(wscr[:], wu[0:1, 0:16])
        nc.sync.dma_start(d_warm.ap(), wscr[:])
        dmyt = ps_g.tile([128, 384], F32, name="Pd", tag="Pd", bufs=1)
        # transpose initial C into CT (cell state kept in transposed layout)
        ptc = ps_tr.tile([128, 128], BF, name="trp0", tag="trp")
        nc.tensor.transpose(ptc[:], C[:], i128t[:])
        CT = lstm_pool.tile([128, 128], BF)
        nc.vector.tensor_copy(CT[:], ptc[:])

        for t in range(T):
            s_idx = 0 if t == 0 else 1
            SA = outA[:, 16 * t:16 * t + 16]
            SB = outB[:, 16 * t:16 * t + 16]
            # bank 1: (i, g', f) gate columns — finishes early so the whole
            # sigmoid/DVE chain overlaps bank 2's (o-gate) streams
            P1 = ps_g.tile([128, 384], F32, name="P1", tag="P1", bufs=1)
            for kk in range(4):
                for hc in range(4):
                    nc.tensor.matmul(P1[32 * hc:32 * hc + BC, :],
                                     hTT[:, 32 * kk:32 * kk + 16],
                                     wGt[:, kk, hc, 0:384],
                                     start=(kk == 0), stop=False,
                                     tile_position=(0, 32 * hc))
            for hc in range(4):
                nc.tensor.matmul(P1[32 * hc:32 * hc + BC, :], SA,
                                 mRAt[:, s_idx, hc, 0:384],
                                 start=False, stop=False,
                                 tile_position=(0, 32 * hc))
            for hc in range(4):
                nc.tensor.matmul(P1[32 * hc:32 * hc + BC, :], SB,
                                 mRBt[:, s_idx, hc, 0:384],
                                 start=False, stop=True,
                                 tile_position=(0, 32 * hc))
            # bank 2: (o)
            P2 = ps_g.tile([128, 128], F32, name="P2", tag="P2", bufs=1)
            for kk in range(4):
                for hc in range(4):
                    nc.tensor.matmul(P2[32 * hc:32 * hc + BC, :],
                                     hTT[:, 32 * kk:32 * kk + 16],
                                     wGt[:, kk, hc, 384:512],
                                     start=(kk == 0), stop=False,
                                     tile_position=(0, 32 * hc))
            for hc in range(4):
                nc.tensor.matmul(P2[32 * hc:32 * hc + BC, :], SA,
                                 mRAt[:, s_idx, hc, 384:512],
                                 start=False, stop=False,
                                 tile_position=(0, 32 * hc))
            for hc in range(4):
                nc.tensor.matmul(P2[32 * hc:32 * hc + BC, :], SB,
                                 mRBt[:, s_idx, hc, 384:512],
                                 start=False, stop=True,
                                 tile_position=(0, 32 * hc))

            # elementwise tail in transposed space; sifo cols (i, g', f, o)
            sifo = work_pool.tile([128, 512], BF, name="sifo", tag="sifo")
            nc.scalar.activation(sifo[:, 0:384], P1[:], Sigmoid)
            nc.scalar.activation(sifo[:, 384:512], P2[:], Sigmoid)
            fT = ps_tr.tile([128, 128], BF, name="fT", tag="fT", bufs=1)
            nc.tensor.transpose(fT[:], sifo[:, 256:384], i128t[:])
            t2 = work_pool.tile([128, 128], BF, name="t2", tag="t2")
            nc.vector.scalar_tensor_tensor(t2[:], sifo[:, 128:256], 0.5,
                                           sifo[:, 0:128],
                                           ALU.subtract, ALU.mult)
            t2T = ps_tr.tile([128, 128], BF, name="t2T", tag="t2T", bufs=1)
            nc.tensor.transpose(t2T[:], t2[:], i128t[:])
            oT = ps_tr.tile([128, 128], BF, name="oT", tag="oT", bufs=1)
            nc.tensor.transpose(oT[:], sifo[:, 384:512], i128t[:])
            # HAM keep-warm filler: dummy rounds on the otherwise-idle PE while
            # the ACT/DVE chain runs (dedicated psum bank, read once after the
            # loop so it is not DCE'd)
            for r in range(5):
                for hc in range(4):
                    nc.tensor.matmul(dmyt[32 * hc:32 * hc + BC, :], hTT[:, 0:16],
                                     wGt[:, r % 4, hc, 0:384],
                                     start=(r == 0), stop=(r == 4),
                                     tile_position=(0, 32 * hc))
            u = work_pool.tile([128, 128], BF, name="u", tag="u")
            nc.vector.tensor_mul(u[:], fT[:], CT[:])
            nc.vector.scalar_tensor_tensor(CT[:], t2T[:], 2.0, u[:],
                                           ALU.mult, ALU.add)
            tch = work_pool.tile([128, 128], BF, name="tch", tag="tch")
            nc.scalar.activation(tch[:], CT[:], Tanh)
            hTT = state_pool.tile([128, 128], BF, name="hTT", tag="hTT")
            nc.vector.tensor_mul(hTT[:, 0:64], oT[:, 0:64], tch[:, 0:64])
            nc.vector.tensor_mul(hTT[:, 64:128], oT[:, 64:128], tch[:, 64:128])

            # head: lf0 (cols 0:16) + uv pre-act (cols 16:32), both partition 0
            phd = ps_hd.tile([1, 32], F32, name="phd", tag="phd")
            for kk in range(4):
                nc.tensor.matmul(phd[0:1, 0:16], hwTt[:, kk, 0:1],
                                 hTT[:, 32 * kk:32 * kk + 16],
                                 start=(kk == 0), stop=(kk == 3))
            for kk in range(4):
                nc.tensor.matmul(phd[0:1, 16:32], hwTt[:, kk, 1:2],
                                 hTT[:, 32 * kk:32 * kk + 16],
                                 start=(kk == 0), stop=(kk == 3))
            o0 = 16 * (t + 1)
            nc.vector.tensor_scalar_add(outA[0:1, o0:o0 + 16], phd[0:1, 0:16],
                                        hb2t[0:1, 0:1])
            nc.scalar.activation(outB[0:1, o0:o0 + 16], phd[0:1, 16:32], Sigmoid,
                                 bias=hb2t[0:1, 1:2])

        wscr2 = work_pool.tile([1, 16], F32, name="wscr2", tag="wscr")
        nc.vector.tensor_copy(wscr2[:], dmyt[0:1, 0:16])
        nc.sync.dma_start(d_warm2.ap(), wscr2[:])
        OFl = lstm_pool.tile([1, T, 16], F32)
        nc.scalar.copy(OFl[:], outA[0:1, 16:16 * (T + 1)].rearrange("p (t b) -> p t b", t=T))
        OFu = lstm_pool.tile([1, T, 16], F32)
        nc.scalar.copy(OFu[:], outB[0:1, 16:16 * (T + 1)].rearrange("p (t b) -> p t b", t=T))
        nc.sync.dma_start(d_out.ap()[0:1], OFl[:])
        nc.sync.dma_start(d_out.ap()[1:2], OFu[:])

    nc.compile()
    return nc


# --------------------------------------------------------------------------
# entry point
# --------------------------------------------------------------------------

def _in_maps(P, T):
    shared = {k: P[k] for k in ["w0T", "cb0", "w1p", "cb1", "w2T", "cb2", "w3T", "cb3",
                                "w4R", "cb4", "pwT", "pb", "wG", "mRA", "mRB", "hwT",
                                "hb2", "i128"]}
    oinitA = np.zeros((2, 16 * (T + 1)), BF_NP)
    oinitA[1, :] = 1.0
    shared["oinitA"] = oinitA
    shared["oinitB"] = np.zeros((1, 16 * (T + 1)), BF_NP)
    in_maps = []
    for c in range(NCORES):
        m = dict(shared)
        m["t0"] = np.ascontiguousarray(P["t0_full"][:, BC * c:BC * c + BC, :])
        in_maps.append(m)
    return in_maps


def kernel(**inputs):
    T = int(np.asarray(inputs["num_steps"]))
    if T not in _CACHE:
        _CACHE[T] = _build(T)
    nc = _CACHE[T]
    P = _prep(inputs)
    in_maps = _in_maps(P, T)
    res = run_bass_kernel_spmd(nc, in_maps, list(range(NCORES)))
    out = np.empty((128, T, 2), np.float32)
    for c in range(NCORES):
        out[BC * c:BC * c + BC] = res.results[c]["out"].transpose(2, 1, 0)
    return out
